# revision 46
# baseline (speedup 1.0000x reference)
"""Multi-head attention (B=2, N=2048, D=2048, 16 heads) on 8 NeuronCores.

Sharding: tensor-parallel over heads (2 heads/core) for QKV projections and
attention; one AllToAll per (head, batch) quarter re-shards the attention
context from head-split to row-split; the output projection is row-parallel
(512 rows/core) with the full Wo on every core.

v2 layout/speed strategy (vs the fp32r baseline):
  - all matmul operands are bf16 (PSUM accumulation stays fp32): halves DMA
    and SBUF traffic at identical PE rate; measured end-to-end error ~1e-3
    against the 2e-2 gate
  - softmax denominators no longer ride the PE as ones-vector matmuls per
    key chunk (that cost as much as the PV matmuls).  Instead the exp chunks
    are summed on DVE with a 4-level pairwise tree, then ONE 512-row
    ones-matmul per query chunk contracts the 128 partitions
  - exp runs double-width: score matmul pairs write a [P,2,512] PSUM tile
    (two banks) and a single ACT instruction exponentiates 1024 columns
  - 1/d is computed on the [1,512] row via reciprocal_approx_fast and
    broadcast across partitions by GPSIMD (no DRAM bounce, no [128,512]
    reciprocal)
  - the output projection is decomposed into half-groups (jc, batch, r2,
    head-half) that are emitted interleaved with the attention quarters:
    h0 halves accumulate in PSUM and stash to SBUF while later quarters'
    attention runs; h1 halves add the stash back during quarter 4.  Only
    batch-1's h1 half trails the last AllToAll.  Dummy matmuls keep the PE
    activity monitor warm across that collective wait.
  - v-bias and o-bias commute out of the kernel: attention rows sum to 1, so
    out = attn@(v0+bv)@Wo.T + bo = device_out + (Wo@bv + bo); host adds it.
"""

import numpy as np
import ml_dtypes

import concourse.bacc as bacc
import concourse.bass_isa as bass_isa
import concourse.mybir as mybir
import concourse.tile as tile
from concourse.bass_utils import run_bass_kernel_spmd

P = 128          # partitions
B = 2            # batch
SEQ = 2048       # sequence length
D = 2048         # hidden
H = 16           # heads
HD = D // H      # head dim = 128
W = 8            # cores
HPC = H // W     # heads per core = 2
DPC = HPC * HD   # features per core = 256
RPC = B * SEQ // W   # rows per core after re-shard = 512
FC = D // P      # feature chunks = 16
RT = B * SEQ     # total rows = 4096
KRC = SEQ // P   # key-row chunks per batch = 16
QRC = SEQ // 512  # query chunks of 512 per batch = 4
HB = RPC // B    # rows per core per batch = 256

f32 = mybir.dt.float32
f32r = mybir.dt.float32r
bf16 = mybir.dt.bfloat16

INV_SQRT_HD = 1.0 / float(np.sqrt(HD))
Act = mybir.ActivationFunctionType
Alu = mybir.AluOpType

_CACHED_NC = None


def build_nc():
    nc = bacc.Bacc("TRN2", target_bir_lowering=False, debug=False)

    xT = nc.dram_tensor("xT", [D, RT], bf16, kind="ExternalInput")
    wqT = nc.dram_tensor("wqT", [D, DPC], bf16, kind="ExternalInput")
    wkT = nc.dram_tensor("wkT", [D, DPC], bf16, kind="ExternalInput")
    wvT = nc.dram_tensor("wvT", [D, DPC], bf16, kind="ExternalInput")
    bq = nc.dram_tensor("bq", [DPC], f32, kind="ExternalInput")
    bk = nc.dram_tensor("bk", [DPC], f32, kind="ExternalInput")
    woT = nc.dram_tensor("woT", [D, D], bf16, kind="ExternalInput")
    ones = nc.dram_tensor("ones", [P, 2], f32r, kind="ExternalInput")
    # out rows: [0:256] = this core's batch-0 rows, [256:512] = batch-1 rows
    out = nc.dram_tensor("out", [RPC, D], f32, kind="ExternalOutput")

    with tile.TileContext(nc) as tc:
        with (
            tc.tile_pool(name="persist", bufs=1) as persist,
            tc.tile_pool(name="dram", bufs=1, space="DRAM") as dram,
        ):
            # ---- persistent SBUF state ----
            qT_sb = persist.tile([P, HPC, RT], bf16)      # [hd, h, row]
            kT_sb = persist.tile([P, HPC, RT], bf16)
            v_sb = persist.tile([P, RT // P, DPC], bf16)  # [row%128, chunk, d]
            bq_sb = persist.tile([P, HPC], f32)
            bk_sb = persist.tile([P, HPC], f32)
            ones_sb = persist.tile([P, 2], f32r)
            bridge = persist.tile([P, 512], bf16)
            wjunk = persist.tile([P, 4], bf16)  # never written: PE warmup
                                                # reads need no DMA

            a2a_in = [[dram.tile([W, HD, HB], bf16, name=f"a2a_in{h}{b}")
                       for b in range(B)] for h in range(HPC)]
            a2a_out = [[dram.tile([W, HD, HB], bf16, name=f"a2a_out{h}{b}")
                        for b in range(B)] for h in range(HPC)]
            wa_in = dram.tile([W, 16], bf16, name="wa_in")
            wa_out = dram.tile([W, 16], bf16, name="wa_out")

            nc.sync.dma_start(ones_sb[:], ones.ap())
            nc.sync.dma_start(bq_sb[:], bq.ap().rearrange("(h p) -> p h", p=P))
            nc.sync.dma_start(bk_sb[:], bk.ap().rearrange("(h p) -> p h", p=P))

            # warm up the collective stream: the first CC op pays ~35us of
            # stream setup + slow-start; burn it on 32 bytes during the
            # QKV phase instead of on the first real AllToAll
            nc.gpsimd.collective_compute(
                "AllToAll", Alu.bypass, replica_groups=[list(range(W))],
                ins=[wa_in[:]], outs=[wa_out[:]])
            # engine-local init (no DMA dependency) so PE warmup can start
            # the moment the queues come up
            nc.vector.memset(wjunk[:], 1.0)

            # ---- HAM warmup: tiny matmuls so the PE clock gate opens
            # before the real work arrives ----
            with tc.tile_pool(name="warm_ps", bufs=1, space="PSUM") as warm_ps:
                wtile = warm_ps.tile([1, 4], f32, name="warm")
                for i in range(32):
                    nc.tensor.matmul(wtile[:], wjunk[:, 0:1],
                                     wjunk[:], start=True, stop=True)

            # ---- phase 1: QKV projections ----
            with (
                tc.tile_pool(name="wproj", bufs=1) as wproj,
                tc.tile_pool(name="xtp", bufs=8) as xtp,
                tc.tile_pool(name="proj_ps", bufs=1, space="PSUM") as proj_ps,
            ):
                wq_sb = wproj.tile([P, FC, DPC], bf16)
                wk_sb = wproj.tile([P, FC, DPC], bf16)
                wv_sb = wproj.tile([P, FC, DPC], bf16)
                # weight chunks spread across three DMA queues so no queue's
                # descriptor-issue rate (~1.66/us) falls behind consumption;
                # wk interleaves with the xt stream on sync inside rc 0
                for fc in range(FC):
                    nc.scalar.dma_start(
                        wq_sb[:, fc, :], wqT.ap()[fc * P:(fc + 1) * P, :])
                    nc.gpsimd.dma_start(
                        wv_sb[:, fc, :], wvT.ap()[fc * P:(fc + 1) * P, :])
                for rc in range(RT // 512):  # 8 row chunks of 512
                    q_ps = [proj_ps.tile([P, 512], f32, tag=f"q{i}", name=f"q_ps{i}")
                            for i in range(HPC)]
                    k_ps = [proj_ps.tile([P, 512], f32, tag=f"k{i}", name=f"k_ps{i}")
                            for i in range(HPC)]
                    v_ps = [proj_ps.tile([P, DPC], f32, tag=f"v{i}", name=f"v_ps{i}")
                            for i in range(4)]
                    for fc in range(FC):
                        if rc == 0:
                            nc.sync.dma_start(
                                wk_sb[:, fc, :],
                                wkT.ap()[fc * P:(fc + 1) * P, :])
                        xt = xtp.tile([P, 512], bf16, tag="xt")
                        nc.sync.dma_start(
                            xt[:],
                            xT.ap()[fc * P:(fc + 1) * P,
                                    rc * 512:(rc + 1) * 512])
                        st = fc == 0
                        sp = fc == FC - 1
                        # interleave short-stream V matmuls between long
                        # Q/K streams so each V LDWEIGHTS hides behind a
                        # 512-cycle stream
                        for i in range(HPC):
                            nc.tensor.matmul(
                                q_ps[i][:], wq_sb[:, fc, i * HD:(i + 1) * HD],
                                xt[:], start=st, stop=sp)
                            nc.tensor.matmul(
                                v_ps[2 * i][:],
                                xt[:, 2 * i * P:(2 * i + 1) * P],
                                wv_sb[:, fc, :], start=st, stop=sp)
                            nc.tensor.matmul(
                                k_ps[i][:], wk_sb[:, fc, i * HD:(i + 1) * HD],
                                xt[:], start=st, stop=sp)
                            nc.tensor.matmul(
                                v_ps[2 * i + 1][:],
                                xt[:, (2 * i + 1) * P:(2 * i + 2) * P],
                                wv_sb[:, fc, :], start=st, stop=sp)
                    # PSUM -> SBUF; Q/K on ACT (with bias), V on DVE
                    for i in range(HPC):
                        nc.scalar.activation(
                            qT_sb[:, i, rc * 512:(rc + 1) * 512], q_ps[i][:],
                            Act.Identity, bias=bq_sb[:, i:i + 1])
                        nc.scalar.activation(
                            kT_sb[:, i, rc * 512:(rc + 1) * 512], k_ps[i][:],
                            Act.Identity, bias=bk_sb[:, i:i + 1])
                    for s4 in range(4):
                        nc.vector.tensor_copy(
                            v_sb[:, rc * 4 + s4, :], v_ps[s4][:])

            # ---- SBUF pools shared by attention + output projection ----
            wo_pool = tc.tile_pool(name="wo", bufs=32)
            wop = wo_pool.__enter__()
            wo_tiles = {}

            def wo_load(jc, hh, i):
                t = wop.tile([P, 512], bf16, tag="wo",
                             name=f"wo_{jc}_{hh}_{i}")
                nc.sync.dma_start(
                    t[:],
                    woT.ap()[i * DPC + hh * HD:i * DPC + (hh + 1) * HD,
                             jc * 512:(jc + 1) * 512])
                wo_tiles[(jc, hh, i)] = t

            ctxl_pool = tc.tile_pool(name="ctxl", bufs=1)
            ctxlp = ctxl_pool.__enter__()
            ctxl = [[ctxlp.tile([P, W, HB], bf16, name=f"ctxl{h}{b}")
                     for b in range(B)] for h in range(HPC)]

            stash_pool = tc.tile_pool(name="stash", bufs=1)
            stashp = stash_pool.__enter__()
            stash = {}
            for jc in range(D // 512):
                for bb in range(B):
                    for r2 in range(HB // P):
                        stash[(jc, bb, r2)] = stashp.tile(
                            [P, 512], f32, name=f"stash_{jc}_{bb}_{r2}")

            osb_pool = tc.tile_pool(name="osb", bufs=3)
            osbp = osb_pool.__enter__()

            # ---- phase 2: attention + interleaved output projection ----
            with (
                tc.tile_pool(name="pt", bufs=2) as ptp,
                tc.tile_pool(name="tree", bufs=1) as treep,
                tc.tile_pool(name="norm", bufs=2) as normp,
                tc.tile_pool(name="accp", bufs=2) as accp,
                tc.tile_pool(name="st_ps", bufs=2, space="PSUM") as st_psp,
                tc.tile_pool(name="ctx_ps", bufs=1, space="PSUM") as ctx_psp,
                tc.tile_pool(name="op_ps", bufs=1, space="PSUM") as op_psp,
            ):
                def half_group(jc, bb, r2, hh, psp):
                    """One output-projection half-group: accumulate 8 source
                    cores' contribution of head-half hh for out rows
                    (bb, r2), cols jc.  Returns 8 single-matmul closures so
                    the caller can interleave them into attention bubbles."""
                    state = {}

                    def mk(i):
                        def f():
                            if i == 0:
                                state['op'] = psp.tile(
                                    [P, 512], f32, tag="op",
                                    name=f"op_{jc}_{bb}_{r2}_{hh}")
                            op = state['op']
                            nc.tensor.matmul(
                                op[:],
                                ctxl[hh][bb][:, i, r2 * P:(r2 + 1) * P],
                                wo_tiles[(jc, hh, i)][:],
                                start=(i == 0), stop=(i == W - 1))
                            if i < W - 1:
                                return
                            if hh == 0:
                                nc.scalar.activation(
                                    stash[(jc, bb, r2)][:], op[:], Act.Copy)
                            else:
                                o_sb = osbp.tile([P, 512], f32, tag="osb")
                                nc.vector.tensor_add(
                                    o_sb[:], op[:], stash[(jc, bb, r2)][:])
                                # alternate DMA queues so the final drain
                                # isn't serialized on one queue's
                                # descriptor-issue rate
                                q = nc.sync if (jc + r2) % 2 == 0 else nc.scalar
                                q.dma_start(
                                    out.ap()[(bb * 2 + r2) * P:
                                             (bb * 2 + r2 + 1) * P,
                                             jc * 512:(jc + 1) * 512],
                                    o_sb[:])
                        return f

                    return [mk(i) for i in range(W)]

                # out-projection micro-ops that may be emitted anywhere
                # (their AllToAll completed at least a full quarter ago)
                ready = []

                def pop_jobs(n):
                    for _ in range(n):
                        if ready:
                            ready.pop(0)()

                # per-qc pop counts at the 4 interleave slots.  qc0 runs
                # only the previous quarter's leftovers (8); this quarter's
                # fresh jobs unlock at qc1, giving the AllToAll that fired
                # at quarter start ~19us before the first dependent matmul
                # (A2A latency varies 8-15us run to run).
                POP_SCHED = {0: (2, 2, 2, 2), 1: (0, 0, 0, 8),
                             2: (6, 6, 6, 6), 3: (6, 6, 6, 6)}
                # the final quarter keeps 24 extra jobs back: together with
                # the previous quarter's leftovers they bridge the last
                # AllToAll with real work instead of dummies
                POP_SCHED_LAST = {0: (2, 2, 2, 2), 1: (0, 0, 0, 8),
                                  2: (6, 6, 6, 6), 3: (2, 2, 2, 2)}

                def attn_quarter(h, b, fresh, last=False):
                    last_ctxn = None
                    for qc in range(QRC):
                        if qc == 1 and fresh:
                            ready.extend(fresh)
                            fresh = []
                        pops = (POP_SCHED_LAST if last else POP_SCHED)[qc]
                        pt = ptp.tile([P, KRC, 512], bf16, tag="pt")
                        ctx_ps = ctx_psp.tile([P, 512], f32, tag="ctx")

                        def s_group(g):
                            # scores for 3 key chunks (1 for the ragged
                            # last group) into one PSUM tile, then ONE
                            # triple-width exp — fewer ACT instructions
                            # amortize the per-instruction bubble that
                            # paces the whole attention phase
                            n = 3 if g < 5 else 1
                            st = st_psp.tile([P, 3, 512], f32, tag="st")
                            for j in range(n):
                                kc = 3 * g + j
                                nc.tensor.matmul(
                                    st[:, j, :],
                                    kT_sb[:, h, b * SEQ + kc * P:
                                          b * SEQ + (kc + 1) * P],
                                    qT_sb[:, h, b * SEQ + qc * 512:
                                          b * SEQ + (qc + 1) * 512],
                                    start=True, stop=True)
                            nc.scalar.activation(
                                pt[:, 3 * g:3 * g + n, :], st[:, 0:n, :],
                                Act.Exp, scale=INV_SQRT_HD)

                        def pv_group(g):
                            n = 3 if g < 5 else 1
                            for j in range(n):
                                kc = 3 * g + j
                                nc.tensor.matmul(
                                    ctx_ps[:],
                                    v_sb[:, b * KRC + kc,
                                         h * HD:(h + 1) * HD],
                                    pt[:, kc, :],
                                    start=(kc == 0), stop=(kc == KRC - 1))

                        s_group(0)
                        s_group(1)
                        pv_group(0)
                        for g in range(2, 6):
                            s_group(g)
                            pop_jobs(pops[g - 2])
                            pv_group(g - 1)
                        pv_group(5)
                        # denominator: 4-level DVE tree, then GPSIMD sums
                        # across partitions (broadcast included for free)
                        t8 = treep.tile([P, 8, 512], bf16, tag="t8")
                        nc.vector.tensor_add(t8[:], pt[:, 0:8, :], pt[:, 8:16, :])
                        t4 = treep.tile([P, 4, 512], bf16, tag="t4")
                        nc.vector.tensor_add(t4[:], t8[:, 0:4, :], t8[:, 4:8, :])
                        t2 = treep.tile([P, 2, 512], bf16, tag="t2")
                        nc.vector.tensor_add(t2[:], t4[:, 0:2, :], t4[:, 2:4, :])
                        acc = accp.tile([P, 512], f32, tag="acc")
                        nc.vector.tensor_add(acc[:], t2[:, 0], t2[:, 1])
                        bc = normp.tile([P, 512], f32, tag="bc")
                        nc.gpsimd.partition_all_reduce(
                            bc[:], acc[:], P, bass_isa.ReduceOp.add)
                        rcp = normp.tile([P, 512], f32, tag="rcp")
                        nc.vector.reciprocal_approx_fast(rcp[:], bc[:])
                        ctxn = normp.tile([P, 512], bf16, tag="ctxn")
                        nc.vector.tensor_tensor(
                            ctxn[:], ctx_ps[:], rcp[:], Alu.mult)
                        last_ctxn = ctxn
                        for s2 in range(2):
                            nc.gpsimd.dma_start(
                                a2a_in[h][b][2 * qc + s2, :, :],
                                ctxn[:, s2 * HB:(s2 + 1) * HB])
                    nc.gpsimd.collective_compute(
                        "AllToAll", Alu.bypass,
                        replica_groups=[list(range(W))],
                        ins=[a2a_in[h][b][:]], outs=[a2a_out[h][b][:]])
                    for i in range(W):
                        nc.gpsimd.dma_start(
                            ctxl[h][b][:, i, :], a2a_out[h][b][i, :, :])
                    return last_ctxn

                def jobs(bb, hh, psp):
                    micro = []
                    for jc in range(D // 512):
                        for r2 in range(HB // P):
                            micro.extend(half_group(jc, bb, r2, hh, psp))
                    return micro

                # quarter 1: heads-0, batch-0 attention (wo h0 prefetch rides
                # the now-idle sync DMA queue)
                for jc in range(D // 512):
                    for i in range(W):
                        wo_load(jc, 0, i)
                attn_quarter(0, 0, [])
                # quarter 2: (h0, b1) attention + out-proj h0-half for b0
                attn_quarter(0, 1, jobs(0, 0, op_psp))
                # quarter 3: (h1, b0) attention + out-proj h0-half for b1
                attn_quarter(1, 0, jobs(1, 0, op_psp))
                # wo h1 tiles replace h0 tiles as their readers retire
                for jc in range(D // 512):
                    for i in range(W):
                        wo_load(jc, 1, i)
                # quarter 4: (h1, b1) attention + out-proj completion for b0
                q4_ctxn = attn_quarter(1, 1, jobs(0, 1, op_psp), last=True)
                # leftover completion jobs (gated on the long-finished
                # third AllToAll) soak up the start of the final A2A wait
                pop_jobs(len(ready))
                # late-gated copy: depends on quarter 4's final ctxn, so it
                # (and the bridge matmuls reading it) cannot be scheduled
                # before the quarter ends
                nc.vector.tensor_copy(bridge[:], q4_ctxn[:])

            # ---- tail: bridge the last AllToAll with dummy matmuls gated
            # on quarter 4's final ctxn (written moments before the A2A
            # trigger, so the scheduler cannot hoist them into earlier
            # bubbles; they also keep the HAM clock gate open), then
            # complete the b1 output projection ----
            with (
                tc.tile_pool(name="warm2", bufs=1, space="PSUM") as warm2,
                tc.tile_pool(name="op2_ps", bufs=4, space="PSUM") as op2_psp,
            ):
                w2 = warm2.tile([1, 512], f32, name="w2")
                for i in range(72):
                    nc.tensor.matmul(w2[:], bridge[:, 0:1], bridge[:],
                                     start=True, stop=True)
                for job in jobs(1, 1, op2_psp):
                    job()
            osb_pool.__exit__(None, None, None)
            stash_pool.__exit__(None, None, None)
            ctxl_pool.__exit__(None, None, None)
            wo_pool.__exit__(None, None, None)

    nc.compile()
    return nc


def kernel(x, Wq, bq, Wk, bk, Wv, bv, Wo, bo, _run_kwargs=None):
    global _CACHED_NC
    if _CACHED_NC is None:
        _CACHED_NC = build_nc()
    nc = _CACHED_NC

    bft = ml_dtypes.bfloat16
    x = np.asarray(x, dtype=np.float32)
    Wq = np.asarray(Wq, dtype=np.float32)
    Wk = np.asarray(Wk, dtype=np.float32)
    Wv = np.asarray(Wv, dtype=np.float32)
    Wo = np.asarray(Wo, dtype=np.float32)
    bq = np.asarray(bq, dtype=np.float32)
    bk = np.asarray(bk, dtype=np.float32)
    bv = np.asarray(bv, dtype=np.float32)
    bo = np.asarray(bo, dtype=np.float32)

    xT = np.ascontiguousarray(x.reshape(RT, D).T).astype(bft)   # [D, RT]
    woT = np.ascontiguousarray(Wo.T).astype(bft)                # [D, D]
    ones = np.ones((P, 2), dtype=np.float32)
    bo_eff = (bo + Wo @ bv).astype(np.float32)                  # [D]

    in_maps = []
    for i in range(W):
        sl = slice(i * DPC, (i + 1) * DPC)
        in_maps.append({
            "xT": xT,
            "wqT": np.ascontiguousarray(Wq[sl, :].T).astype(bft),
            "wkT": np.ascontiguousarray(Wk[sl, :].T).astype(bft),
            "wvT": np.ascontiguousarray(Wv[sl, :].T).astype(bft),
            "bq": np.ascontiguousarray(bq[sl]),
            "bk": np.ascontiguousarray(bk[sl]),
            "woT": woT,
            "ones": ones,
        })

    kw = _run_kwargs or {}
    res = run_bass_kernel_spmd(nc, in_maps, core_ids=list(range(W)), **kw)

    full = np.empty((RT, D), dtype=np.float32)
    for i in range(W):
        o = res.results[i]["out"]
        full[i * HB:(i + 1) * HB, :] = o[:HB]              # batch 0 rows
        full[SEQ + i * HB:SEQ + (i + 1) * HB, :] = o[HB:]  # batch 1 rows
    full += bo_eff[None, :]
    out = full.reshape(B, SEQ, D)
    if kw:
        kernel.last_results = res
    return out
